# revision 1
# baseline (speedup 1.0000x reference)
"""Trainium2 Bass kernel for nn_CorrBlockSingleScale (RAFT single-scale
correlation lookup), distributed over 8 NeuronCores.

  fmap1, fmap2: [1, 256, 64, 96] f32;  coords: [1, 2, 64, 96] f32; radius=4
  corr = einsum('bcm,bcn->bmn', f1, f2) / 16        -> [6144, 64, 96]
  out[q, i, j] = bilinear(corr[q], (cx_q + d_i, cy_q + d_j)),  d in -4..4
  output [1, 81, 64, 96] f32.

Structure exploited: the 9x9 sample offsets are integers, so all 81 samples
of a query share one fractional pair (fx, fy) -- the output is a separable
2x2-tap blend of a 10x10 patch of corr[q] anchored at
(floor(cx)-4, floor(cy)-4).

Distribution (no collectives): queries are sorted by floor(cy) on the host;
each core takes 768 contiguous sorted queries and therefore only needs a
narrow y-band (~19 of 64 rows) of the correlation target plane.  Per core:
  1. matmul f1_tile^T @ f2_band with K=256 split into bf16 hi/lo pairs
     (3 accumulating matmuls per K-half: hi*hi, hi*lo, lo*hi -- fp32-class
     accuracy at bf16 PE throughput).  Band columns are host-permuted to
     x-major order so each query's corr band lands transposed in DRAM.
  2. DMA the band to a per-tile DRAM scratch slot per query.
  3. indirect-DMA gather one contiguous window per query (the 10x10 patch
     spans 9*W_ROWS+10 elements in the x-major layout).
  4. blend the patch with host-folded bilinear weights + validity masks on
     the vector engine; DMA out [128, 81] rows.
Host post-pass inverse-permutes and transposes to the reference layout.
"""


import numpy as np

import concourse.bass as bass
import concourse.bacc as bacc
import concourse.mybir as mybir
import concourse.tile as tile
from concourse import bass_utils
from concourse.bass import ts

F32 = mybir.dt.float32
I32 = mybir.dt.int32

B, C, H, W = 1, 256, 64, 96
R = 4
K = 2 * R + 1          # 9
PK = K + 1             # 10 (patch side)
NQ = H * W             # 6144
NCORES = 8
QPC = NQ // NCORES     # 768
P = 128
NT = QPC // P          # 6 tiles per core
GUARD = 512            # head guard (window can start below the slot)
GUARD_TAIL = 1024      # tail guard (window can end past the last slot)


# --------------------------------------------------------------------------
# host-side preprocessing
# --------------------------------------------------------------------------

def host_preprocess(fmap1, fmap2, coords):
    """Returns (in_maps, order, NF)."""
    f1 = np.asarray(fmap1, np.float32).reshape(C, NQ)
    f2 = np.asarray(fmap2, np.float32).reshape(C, NQ)
    cx = np.asarray(coords, np.float32)[0, 0].reshape(NQ)
    cy = np.asarray(coords, np.float32)[0, 1].reshape(NQ)

    ix = np.floor(cx)
    iy = np.floor(cy)
    fx = cx - ix          # exact in fp32
    fy = cy - iy
    ixi = ix.astype(np.int64)
    iyi = iy.astype(np.int64)

    order = np.argsort(iyi, kind="stable")

    # uniform band width across cores
    w_req = 0
    for c in range(NCORES):
        qs = order[c * QPC:(c + 1) * QPC]
        w_req = max(w_req, int(iyi[qs].max() - iyi[qs].min()) + PK)
    W_ROWS = min(H, w_req)
    NF = W_ROWS * W

    in_maps = []
    for c in range(NCORES):
        qs = order[c * QPC:(c + 1) * QPC]
        miny = int(iyi[qs].min())
        r0 = int(np.clip(miny - R, 0, H - W_ROWS))

        f1s = f1[:, qs].reshape(2, P, QPC)
        # band columns reordered x-major (c*W_ROWS + r): the corr band then
        # lands in DRAM transposed per query, so a patch window spans only
        # 9*W_ROWS+10 elements instead of 9*96+10.
        f2w = f2[:, r0 * W: r0 * W + NF].reshape(C, W_ROWS, W)
        f2s = np.ascontiguousarray(f2w.transpose(0, 2, 1).reshape(2, P, NF))

        jy = iyi[qs]           # [768]
        jx = ixi[qs]
        a = np.arange(PK)      # [10]
        r_abs = jy[:, None] - R + a[None, :]            # patch row abs y
        # per-query window start (one gather offset per query)
        idx = (GUARD + (np.arange(QPC) % P) * NF
               + (jx - R) * W_ROWS + (jy - R - r0)).astype(np.int32)[:, None]

        bcol = np.arange(PK)
        mx = ((jx[:, None] - R + bcol[None, :] >= 0)
              & (jx[:, None] - R + bcol[None, :] <= W - 1))   # [768,10]
        my = (r_abs >= 0) & (r_abs <= H - 1)                  # [768,10]
        # transposed mask layout [q, b(x), a(y)]
        m2 = (mx[:, :, None] & my[:, None, :]).astype(np.float32)

        wx1 = fx[qs].astype(np.float32)
        wy1 = fy[qs].astype(np.float32)
        # inner (window-minor) axis is y -> inner mix uses wy, outer uses wx
        wts = np.stack([(1.0 - wy1), wy1,
                        (1.0 - wx1) / 16.0, wx1 / 16.0], axis=1).astype(np.float32)

        in_maps.append({
            "f1s": np.ascontiguousarray(f1s),
            "f2s": np.ascontiguousarray(f2s),
            "idx": idx,
            "m2": np.ascontiguousarray(m2.reshape(QPC, PK * PK)),
            "wts": np.ascontiguousarray(wts),
        })
    return in_maps, order, NF


def split_bf16_inputs(in_maps):
    """Replace f1s/f2s with bf16 hi/lo splits (for mm_dtype='bf16x3')."""
    import ml_dtypes
    bf16 = ml_dtypes.bfloat16
    out = []
    for m in in_maps:
        m = dict(m)
        for name in ("f1s", "f2s"):
            x = m.pop(name).astype(np.float32)
            hi = x.astype(bf16)
            lo = (x - hi.astype(np.float32)).astype(bf16)
            m[name + "h"] = hi
            m[name + "l"] = lo
        out.append(m)
    return out


def assemble_output(results, order):
    rows = np.concatenate([results[c]["out"] for c in range(NCORES)], axis=0)
    # device blend emits [dx, dy]-major, matching the reference's 81-axis
    # (delta[..., 0] is added to x and varies along the first grid axis)
    full = np.empty((K * K, NQ), np.float32)
    full[:, order] = rows.T
    return full.reshape(1, K * K, H, W)


# --------------------------------------------------------------------------
# device program
# --------------------------------------------------------------------------

def _body(tc, nc, aps, scr, NF, nchunks, mm_dtype=F32):
    idx, m2, wts, out = aps["idx"], aps["m2"], aps["wts"], aps["out"]
    bf3 = (mm_dtype == "bf16x3")
    import contextlib
    ctx = contextlib.ExitStack()
    with ctx:
        const = ctx.enter_context(tc.tile_pool(name="const", bufs=1))
        corr_pool = ctx.enter_context(tc.tile_pool(name="corr", bufs=2))
        psum_pool = ctx.enter_context(
            tc.tile_pool(name="ps", bufs=4, space="PSUM"))
        small = ctx.enter_context(tc.tile_pool(name="small", bufs=3))

        # resident inputs.  mm_list: (lhsT sbuf tile, rhs sbuf tile, k) per
        # accumulating matmul of one output chunk.
        if bf3:
            BF = mybir.dt.bfloat16
            f1bh = const.tile([P, 2 * QPC], BF)
            f1bl = const.tile([P, 2 * QPC], BF)
            f2bh0 = const.tile([P, NF], BF)
            f2bl0 = const.tile([P, NF], BF)
            f2bh1 = const.tile([P, NF], BF)
            f2bl1 = const.tile([P, NF], BF)
            for k in range(2):
                nc.sync.dma_start(f1bh[:, k * QPC:(k + 1) * QPC],
                                  aps["f1sh"][k])
                nc.sync.dma_start(f1bl[:, k * QPC:(k + 1) * QPC],
                                  aps["f1sl"][k])
            nc.sync.dma_start(f2bh0[:], aps["f2sh"][0])
            nc.sync.dma_start(f2bh1[:], aps["f2sh"][1])
            nc.sync.dma_start(f2bl0[:], aps["f2sl"][0])
            nc.sync.dma_start(f2bl1[:], aps["f2sl"][1])
            f2bh = [f2bh0, f2bh1]
            f2bl = [f2bl0, f2bl1]
            mm_list = [(f1bh, f2bh[0], 0), (f1bh, f2bh[1], 1),
                       (f1bh, f2bl[0], 0), (f1bl, f2bh[0], 0),
                       (f1bh, f2bl[1], 1), (f1bl, f2bh[1], 1)]
        else:
            f1b = const.tile([P, 2 * QPC], F32)
            nc.sync.dma_start(f1b[:, 0:QPC], aps["f1s"][0])
            nc.sync.dma_start(f1b[:, QPC:2 * QPC], aps["f1s"][1])
            f2b0 = const.tile([P, NF], F32)
            nc.sync.dma_start(f2b0[:], aps["f2s"][0])
            f2b1 = const.tile([P, NF], F32)
            nc.sync.dma_start(f2b1[:], aps["f2s"][1])
            f2b = [f2b0, f2b1]
            mm_list = [(f1b, f2b[0], 0), (f1b, f2b[1], 1)]

        idxb = const.tile([P, NT], I32)
        nc.sync.dma_start(idxb[:].rearrange("p (t a) -> p t a", a=1),
                          idx.rearrange("(t p) a -> p t a", p=P))
        m2b = const.tile([P, NT * PK * PK], F32)
        nc.sync.dma_start(m2b[:].rearrange("p (t a) -> p t a", a=PK * PK),
                          m2.rearrange("(t p) a -> p t a", p=P))
        wtsb = const.tile([P, NT * 4], F32)
        nc.sync.dma_start(wtsb[:].rearrange("p (t a) -> p t a", a=4),
                          wts.rearrange("(t p) a -> p t a", p=P))

        chunks = [(i * 512, min(512, NF - i * 512)) for i in range(nchunks)]

        # zero the scratch guard bands (a masked-out window row may read them;
        # uninitialized HBM could hold NaN and 0*NaN would poison the blend)
        zt = const.tile([1, GUARD_TAIL], F32)
        nc.vector.memset(zt[:], 0.0)
        for t in range(NT):
            g = scr[t].ap()[0:GUARD].rearrange("(p f) -> p f", p=1)
            nc.sync.dma_start(g, zt[:, 0:GUARD])
            g = scr[t].ap()[GUARD + P * NF:GUARD + P * NF + GUARD_TAIL] \
                .rearrange("(p f) -> p f", p=1)
            nc.sync.dma_start(g, zt[:])

        for t in range(NT):
            corr_sb = corr_pool.tile([P, NF], F32)
            for ci, (c0, cw) in enumerate(chunks):
                ps = psum_pool.tile([P, 512], F32, space="PSUM", tag="ps")
                for mi, (f1t, f2t, k) in enumerate(mm_list):
                    lhsT = f1t[:, k * QPC + t * P: k * QPC + (t + 1) * P]
                    rhs = f2t[:, c0:c0 + cw]
                    if not bf3 and mm_dtype != F32:
                        lhsT = lhsT.bitcast(mm_dtype)
                        rhs = rhs.bitcast(mm_dtype)
                    nc.tensor.matmul(
                        ps[:, :cw], lhsT=lhsT, rhs=rhs,
                        start=(mi == 0), stop=(mi == len(mm_list) - 1))
                # alternate PSUM->SBUF copies across ACT and DVE
                if ci % 2 == 0:
                    nc.scalar.copy(corr_sb[:, c0:c0 + cw], ps[:, :cw])
                else:
                    nc.vector.tensor_copy(corr_sb[:, c0:c0 + cw], ps[:, :cw])

            dst = scr[t].ap()[GUARD:GUARD + P * NF] \
                .rearrange("(p f) -> p f", p=P)
            nc.sync.dma_start(dst, corr_sb[:])

            wrows = NF // W
            win = (PK - 1) * wrows + PK
            pt = small.tile([P, PK * wrows], F32, tag="pt")
            src = scr[t].ap().rearrange("(n o) -> n o", o=1)
            nc.gpsimd.indirect_dma_start(
                out=pt[:, 0:win], out_offset=None, in_=src,
                in_offset=bass.IndirectOffsetOnAxis(
                    ap=idxb[:, t:t + 1], axis=0))
            # patch view: x-strips at stride wrows inside the gathered window
            ptv = pt[:].rearrange("p (b r) -> p b r", r=wrows)[:, :, 0:PK]

            pm = small.tile([P, PK * PK], F32, tag="pm")
            nc.vector.tensor_tensor(
                pm[:].rearrange("p (a b) -> p a b", b=PK), ptv,
                m2b[:, ts(t, PK * PK)].rearrange("p (a b) -> p a b", b=PK),
                op=mybir.AluOpType.mult)
            pm3 = pm[:].rearrange("p (a b) -> p a b", b=PK)

            t1 = small.tile([P, PK * K], F32, tag="t1")
            t13 = t1[:].rearrange("p (a b) -> p a b", b=K)
            nc.vector.tensor_scalar_mul(
                t13, pm3[:, :, 1:PK], wtsb[:, 4 * t + 1: 4 * t + 2])
            cm = small.tile([P, PK * K], F32, tag="cm")
            cm3 = cm[:].rearrange("p (a b) -> p a b", b=K)
            nc.vector.scalar_tensor_tensor(
                cm3, pm3[:, :, 0:K], wtsb[:, 4 * t: 4 * t + 1], t13,
                op0=mybir.AluOpType.mult, op1=mybir.AluOpType.add)

            t2 = small.tile([P, K * K], F32, tag="t2")
            t23 = t2[:].rearrange("p (a b) -> p a b", b=K)
            nc.vector.tensor_scalar_mul(
                t23, cm3[:, 1:PK, :], wtsb[:, 4 * t + 3: 4 * t + 4])
            ot = small.tile([P, K * K], F32, tag="ot")
            ot3 = ot[:].rearrange("p (a b) -> p a b", b=K)
            nc.vector.scalar_tensor_tensor(
                ot3, cm3[:, 0:K, :], wtsb[:, 4 * t + 2: 4 * t + 3], t23,
                op0=mybir.AluOpType.mult, op1=mybir.AluOpType.add)

            nc.sync.dma_start(out[ts(t, P), :], ot[:])


def build_program(NF, rep=1, mm_dtype=F32):
    """rep>1 wraps the body in a For_i loop (for wall-clock timing)."""
    nchunks = (NF + 511) // 512
    nc = bacc.Bacc("TRN2", target_bir_lowering=False, debug=False,
                   num_devices=NCORES)
    aps = {}
    if mm_dtype == "bf16x3":
        BF = mybir.dt.bfloat16
        for nm in ("f1sh", "f1sl"):
            aps[nm] = nc.dram_tensor(nm, [2, P, QPC], BF,
                                     kind="ExternalInput").ap()
        for nm in ("f2sh", "f2sl"):
            aps[nm] = nc.dram_tensor(nm, [2, P, NF], BF,
                                     kind="ExternalInput").ap()
    else:
        aps["f1s"] = nc.dram_tensor("f1s", [2, P, QPC], F32,
                                    kind="ExternalInput").ap()
        aps["f2s"] = nc.dram_tensor("f2s", [2, P, NF], F32,
                                    kind="ExternalInput").ap()
    aps["idx"] = nc.dram_tensor("idx", [QPC, 1], I32,
                                kind="ExternalInput").ap()
    aps["m2"] = nc.dram_tensor("m2", [QPC, PK * PK], F32,
                               kind="ExternalInput").ap()
    aps["wts"] = nc.dram_tensor("wts", [QPC, 4], F32,
                                kind="ExternalInput").ap()
    aps["out"] = nc.dram_tensor("out", [QPC, K * K], F32,
                                kind="ExternalOutput").ap()
    scr = [nc.dram_tensor(f"scr{t}", [GUARD + P * NF + GUARD_TAIL], F32)
           for t in range(NT)]

    with tile.TileContext(nc) as tc:
        if rep == 1:
            _body(tc, nc, aps, scr, NF, nchunks, mm_dtype)
        else:
            with tc.For_i(0, rep):
                _body(tc, nc, aps, scr, NF, nchunks, mm_dtype)
    nc.compile()
    return nc


_PROGRAMS = {}


def kernel(fmap1, fmap2, coords, radius):
    assert int(radius) == R, f"kernel hardcodes radius=4, got {radius}"
    in_maps, order, NF = host_preprocess(fmap1, fmap2, coords)
    in_maps = split_bf16_inputs(in_maps)
    nc = _PROGRAMS.get(NF)
    if nc is None:
        nc = _PROGRAMS[NF] = build_program(NF, mm_dtype="bf16x3")
    last_err = None
    for _ in range(3):  # the remote compile hook occasionally flakes
        try:
            res = bass_utils.run_bass_kernel_spmd(
                nc, in_maps, core_ids=list(range(NCORES)))
            return assemble_output(res.results, order)
        except Exception as e:  # noqa: BLE001
            last_err = e
    raise last_err



# revision 7
# speedup vs baseline: 1.5675x; 1.5675x over previous
"""Trainium2 Bass kernel for nn_CorrBlockSingleScale (RAFT single-scale
correlation lookup), distributed over 8 NeuronCores.

  fmap1, fmap2: [1, 256, 64, 96] f32;  coords: [1, 2, 64, 96] f32; radius=4
  corr = einsum('bcm,bcn->bmn', f1, f2) / 16        -> [6144, 64, 96]
  out[q, i, j] = bilinear(corr[q], (cx_q + d_i, cy_q + d_j)),  d in -4..4
  output [1, 81, 64, 96] f32.

Structure exploited: the 9x9 sample offsets are integers, so all 81 samples
of a query share one fractional pair (fx, fy) -- the output is a separable
2x2-tap blend of a 10x10 patch of corr[q] anchored at
(floor(cx)-4, floor(cy)-4).

Distribution / tiling (no collectives): queries are sorted by floor(cy) and
chopped into 8 cores of 768; within a core they are re-sorted by floor(cx)
and chopped into 6 tiles of 128.  Each tile's 128 queries therefore cluster
in a ~16x10-cell rectangle, so its correlation band is only
BX_t x SROWS cells (~32 x 18) instead of the full 64x96 target plane.
Per-tile x-anchors are unified across cores (max extent) so one SPMD
program serves all 8 cores.

Per core:
  1. bf16 matmul f1_tile^T @ f2_band (K=256 as 2 accumulating matmuls of
     128) producing the x-major band [128, BX*SROWS] in PSUM.  The f2 slab
     sits in SBUF zero-padded to x in [-5,101), y rows [ys, ys+SROWS), so
     out-of-plane bilinear taps read stored zeros and no masks are needed.
  2. cast band to bf16 (ACT/DVE alternating) and DMA to per-tile DRAM
     scratch; indirect-DMA gather one contiguous window per query (the
     10x10 patch spans 9*SROWS+10 elements of the x-major band).
  3. separable blend with host-folded bilinear weights: two per-partition
     scalar muls on ACT, two scalar_tensor_tensor FMAs on DVE.
  4. one output DMA [768, 81] f32 at the end.
Host post-pass inverse-permutes rows to the reference layout.
"""

import math

import numpy as np
import ml_dtypes

import concourse.bass as bass
import concourse.bacc as bacc
import concourse.mybir as mybir
import concourse.tile as tile
from concourse import bass_utils

BF16NP = ml_dtypes.bfloat16
F32 = mybir.dt.float32
I32 = mybir.dt.int32
BF16 = mybir.dt.bfloat16

B, C, H, W = 1, 256, 64, 96
R = 4
K = 2 * R + 1          # 9
PK = K + 1             # 10 (patch side)
NQ = H * W             # 6144
NCORES = 8
QPC = NQ // NCORES     # 768
P = 128
NT = QPC // P          # 6 tiles per core
KH = 2                 # K halves (256 = 2 x 128)
PADX = 5               # padded x coords [-5, 101)
PADY = 5               # padded y coords [-5, 69)
WPAD = W + 2 * PADX    # 106


# --------------------------------------------------------------------------
# host-side preprocessing
# --------------------------------------------------------------------------

def host_preprocess(fmap1, fmap2, coords):
    """Returns (in_maps, order, geom) with geom = (SROWS, (BX_0..BX_5))."""
    f1 = np.asarray(fmap1, np.float32).reshape(C, NQ)
    f2 = np.asarray(fmap2, np.float32).reshape(C, H, W)
    cx_all = np.asarray(coords, np.float32)[0, 0].reshape(NQ)
    cy_all = np.asarray(coords, np.float32)[0, 1].reshape(NQ)
    ix_all = np.floor(cx_all).astype(np.int64)
    iy_all = np.floor(cy_all).astype(np.int64)

    # core chop by y-quantile, then within-core x sort -> tiles are x-runs
    yorder = np.lexsort((np.arange(NQ), ix_all, iy_all))
    order = np.empty(NQ, np.int64)
    for c in range(NCORES):
        qs = yorder[c * QPC:(c + 1) * QPC]
        sub = np.lexsort((np.arange(QPC), iy_all[qs], ix_all[qs]))
        order[c * QPC:(c + 1) * QPC] = qs[sub]

    # zero-padded bf16 target plane [C, 74, 106]
    plane = np.zeros((C, H + 2 * PADY, WPAD), np.float32)
    plane[:, PADY:PADY + H, PADX:PADX + W] = f2
    plane = plane.astype(BF16NP)

    cores = []
    for c in range(NCORES):
        qs = order[c * QPC:(c + 1) * QPC]
        jx = ix_all[qs]
        jy = iy_all[qs]
        fx = (cx_all[qs] - jx).astype(np.float32)
        fy = (cy_all[qs] - jy).astype(np.float32)
        cores.append(dict(qs=qs, jx=jx, jy=jy, fx=fx, fy=fy))

    SROWS = max(int(c["jy"].max() - c["jy"].min()) + PK for c in cores)
    for c in cores:
        ys = int(c["jy"].min()) - R
        ys = max(min(ys, H + PADY - SROWS), -PADY)
        c["ys"] = ys
        assert ys <= c["jy"].min() - R
        assert ys + SROWS >= c["jy"].max() + R + 2

    ax = []
    BX = []
    for t in range(NT):
        sel = slice(t * P, (t + 1) * P)
        lo = min(int(c["jx"][sel].min()) for c in cores) - R
        hi = max(int(c["jx"][sel].max()) for c in cores) + R + 1
        ax.append(lo)
        BX.append(hi - lo + 1)

    f1b = f1.astype(BF16NP)
    in_maps = []
    for c in cores:
        qs = c["qs"]
        jx, jy = c["jx"], c["jy"]
        ys = c["ys"]
        f1s = np.ascontiguousarray(f1b[:, qs].reshape(KH, P, QPC))
        slab = np.ascontiguousarray(
            plane[:, ys + PADY: ys + PADY + SROWS, :].reshape(KH, P, SROWS * WPAD))

        meta = np.empty((QPC, 5), np.int32)
        for t in range(NT):
            sel = slice(t * P, (t + 1) * P)
            start = ((jx[sel] - R - ax[t]) * SROWS + (jy[sel] - R - ys)
                     + np.arange(P) * (BX[t] * SROWS))
            win = (PK - 1) * SROWS + PK
            assert start.min() >= np.arange(P).min() * BX[t] * SROWS
            assert ((start + win) <= (np.arange(P) + 1) * BX[t] * SROWS).all()
            meta[sel, 0] = start.astype(np.int32)
            wy0 = (1.0 - c["fy"][sel]).astype(np.float32)
            wy1 = c["fy"][sel].astype(np.float32)
            wx0 = ((1.0 - c["fx"][sel]) / 16.0).astype(np.float32)
            wx1 = (c["fx"][sel] / 16.0).astype(np.float32)
            meta[sel, 1] = wy0.view(np.int32)
            meta[sel, 2] = wy1.view(np.int32)
            meta[sel, 3] = wx0.view(np.int32)
            meta[sel, 4] = wx1.view(np.int32)

        in_maps.append({"f1s": f1s, "f2s": slab, "meta": meta})

    geom = (SROWS, tuple(BX), tuple(ax))
    return in_maps, order, geom


def assemble_output(results, order):
    rows = np.concatenate([results[c]["out"] for c in range(NCORES)], axis=0)
    full = np.empty((K * K, NQ), np.float32)
    full[:, order] = rows.T
    return full.reshape(1, K * K, H, W)


# --------------------------------------------------------------------------
# device program
# --------------------------------------------------------------------------

def _body(tc, nc, aps, scr, geom):
    SROWS, BX, ax = geom
    win = (PK - 1) * SROWS + PK
    import contextlib
    ctx = contextlib.ExitStack()
    with ctx:
        const = ctx.enter_context(tc.tile_pool(name="const", bufs=1))
        corr_pool = ctx.enter_context(tc.tile_pool(name="corr", bufs=3))
        psum_pool = ctx.enter_context(
            tc.tile_pool(name="ps", bufs=4, space="PSUM"))
        small = ctx.enter_context(tc.tile_pool(name="small", bufs=3))

        f1sb = const.tile([P, KH * QPC], BF16)
        nc.sync.dma_start(
            f1sb[:].rearrange("p (k m) -> p k m", k=KH),
            aps["f1s"].rearrange("k p m -> p k m"))
        f2sb = const.tile([P, KH * SROWS * WPAD], BF16)
        nc.sync.dma_start(
            f2sb[:].rearrange("p (k m) -> p k m", k=KH),
            aps["f2s"].rearrange("k p m -> p k m"))
        metab = const.tile([P, NT * 5], I32)
        nc.sync.dma_start(
            metab[:].rearrange("p (t a) -> p t a", a=5),
            aps["meta"].rearrange("(t p) a -> p t a", p=P))
        otall = const.tile([P, NT * K * K], F32)

        # [p, kh, x(col, stride 1), r(row, stride WPAD)]
        f2v = f2sb[:].rearrange("p (k r x) -> p k x r", k=KH, x=WPAD)

        cbx_max = max(1, 512 // SROWS)
        for t in range(NT):
            bxt = BX[t]
            nchunk = math.ceil(bxt / cbx_max)
            corr_sb = corr_pool.tile([P, bxt * SROWS], BF16, tag="corr")
            x0 = 0
            ci = 0
            while x0 < bxt:
                cbx = min(cbx_max, bxt - x0)
                cw = cbx * SROWS
                ps = psum_pool.tile([P, 512], F32, space="PSUM", tag="ps")
                for kh in range(KH):
                    lhsT = f1sb[:, kh * QPC + t * P: kh * QPC + (t + 1) * P]
                    rhs = f2v[:, kh, ax[t] + PADX + x0:
                              ax[t] + PADX + x0 + cbx, :]
                    nc.tensor.matmul(ps[:, :cw], lhsT=lhsT, rhs=rhs,
                                     start=(kh == 0), stop=(kh == KH - 1))
                dst = corr_sb[:, x0 * SROWS: x0 * SROWS + cw]
                if ci % 2 == 0:
                    nc.scalar.copy(dst, ps[:, :cw])
                else:
                    nc.vector.tensor_copy(dst, ps[:, :cw])
                x0 += cbx
                ci += 1

            sdst = scr[t].ap()[0: P * bxt * SROWS].rearrange(
                "(p f) -> p f", p=P)
            nc.sync.dma_start(sdst, corr_sb[:])

            pt = small.tile([P, PK * SROWS], BF16, tag="pt")
            src = scr[t].ap().rearrange("(n o) -> n o", o=1)
            nc.gpsimd.indirect_dma_start(
                out=pt[:, 0:win], out_offset=None, in_=src,
                in_offset=bass.IndirectOffsetOnAxis(
                    ap=metab[:, 5 * t: 5 * t + 1], axis=0))
            ptv = pt[:].rearrange("p (b r) -> p b r", r=SROWS)[:, :, 0:PK]

            def wap(a):
                return metab[:, 5 * t + 1 + a: 5 * t + 2 + a].bitcast(F32)

            t1 = small.tile([P, PK * K], F32, tag="t1")
            t13 = t1[:].rearrange("p (a b) -> p a b", b=K)
            nc.scalar.mul(t13, ptv[:, :, 1:PK], wap(1))
            cm = small.tile([P, PK * K], F32, tag="cm")
            cm3 = cm[:].rearrange("p (a b) -> p a b", b=K)
            nc.vector.scalar_tensor_tensor(
                cm3, ptv[:, :, 0:K], wap(0), t13,
                op0=mybir.AluOpType.mult, op1=mybir.AluOpType.add)

            t2 = small.tile([P, K * K], F32, tag="t2")
            t23 = t2[:].rearrange("p (a b) -> p a b", b=K)
            nc.scalar.mul(t23, cm3[:, 1:PK, :], wap(3))
            ot3 = otall[:, t * K * K: (t + 1) * K * K].rearrange(
                "p (a b) -> p a b", b=K)
            nc.vector.scalar_tensor_tensor(
                ot3, cm3[:, 0:K, :], wap(2), t23,
                op0=mybir.AluOpType.mult, op1=mybir.AluOpType.add)

        nc.sync.dma_start(
            aps["out"].rearrange("(t p) k -> p t k", p=P),
            otall[:].rearrange("p (t k) -> p t k", k=K * K))


def build_program(geom, rep=1):
    """rep>1 wraps the body in a For_i loop (for wall-clock timing)."""
    SROWS, BX, ax = geom
    nc = bacc.Bacc("TRN2", target_bir_lowering=False, debug=False,
                   num_devices=NCORES)
    aps = {}
    aps["f1s"] = nc.dram_tensor("f1s", [KH, P, QPC], BF16,
                                kind="ExternalInput").ap()
    aps["f2s"] = nc.dram_tensor("f2s", [KH, P, SROWS * WPAD], BF16,
                                kind="ExternalInput").ap()
    aps["meta"] = nc.dram_tensor("meta", [QPC, 5], I32,
                                 kind="ExternalInput").ap()
    aps["out"] = nc.dram_tensor("out", [QPC, K * K], F32,
                                kind="ExternalOutput").ap()
    scr = [nc.dram_tensor(f"scr{t}", [P * BX[t] * SROWS], BF16)
           for t in range(NT)]

    with tile.TileContext(nc) as tc:
        if rep == 1:
            _body(tc, nc, aps, scr, geom)
        else:
            with tc.For_i(0, rep):
                _body(tc, nc, aps, scr, geom)
    nc.compile()
    return nc


_PROGRAMS = {}


def kernel(fmap1, fmap2, coords, radius):
    assert int(radius) == R, f"kernel hardcodes radius=4, got {radius}"
    in_maps, order, geom = host_preprocess(fmap1, fmap2, coords)
    nc = _PROGRAMS.get(geom)
    if nc is None:
        nc = _PROGRAMS[geom] = build_program(geom)
    last_err = None
    for _ in range(3):  # the remote compile hook occasionally flakes
        try:
            res = bass_utils.run_bass_kernel_spmd(
                nc, in_maps, core_ids=list(range(NCORES)))
            return assemble_output(res.results, order)
        except Exception as e:  # noqa: BLE001
            last_err = e
    raise last_err
